# revision 1
# baseline (speedup 1.0000x reference)
"""Trainium kernel for nn_NeSyBase: BiLSTM video/text encoders + MLP routing.

Dev version: imports lstm_kernels (will be inlined for submission).
"""

import time

import numpy as np

import lstm_kernels as L
from concourse.bass_utils import run_bass_kernel_spmd

_CACHE = {}
LAST_PHASE_TIMES = {}


def kernel(**inputs) -> np.ndarray:
    if "d1" not in _CACHE:
        _CACHE["d1"] = L.build_d1()
        _CACHE["d2"] = L.build_d2()
    d1_nc, d2_nc = _CACHE["d1"], _CACHE["d2"]

    vid = np.asarray(inputs["vid_feats"], np.float32).reshape(L.SEQ, L.TV, L.DV)
    txt = np.asarray(inputs["text_feats"], np.float32).reshape(L.SEQ, L.TT, L.DT)

    for bname in ("vid_b_f", "vid_b_b", "txt_b_f", "txt_b_b"):
        assert not np.any(np.asarray(inputs[bname])), (
            f"nonzero LSTM bias {bname} not supported"
        )

    t0 = time.time()
    d1_in = L._prep_d1_inputs(
        vid, txt,
        np.asarray(inputs["vid_Wx_f"]), np.asarray(inputs["vid_Wx_b"]),
        np.asarray(inputs["txt_Wx_f"]), np.asarray(inputs["txt_Wx_b"]),
    )
    LAST_PHASE_TIMES["prep_d1"] = time.time() - t0

    t0 = time.time()
    r1 = run_bass_kernel_spmd(d1_nc, d1_in, list(range(8)))
    LAST_PHASE_TIMES["d1"] = time.time() - t0

    t0 = time.time()
    d2_in = L._assemble_d2_inputs(
        r1.results,
        np.asarray(inputs["vid_Wh_f"]), np.asarray(inputs["vid_Wh_b"]),
        np.asarray(inputs["txt_Wh_f"]), np.asarray(inputs["txt_Wh_b"]),
    )
    LAST_PHASE_TIMES["prep_d2"] = time.time() - t0

    t0 = time.time()
    r2 = run_bass_kernel_spmd(d2_nc, d2_in, list(range(8)))
    LAST_PHASE_TIMES["d2"] = time.time() - t0

    t0 = time.time()
    feats = L._assemble_feats(r2.results)

    def mlp(W1, b1, W2, b2):
        h1 = np.maximum(
            feats @ np.asarray(W1, np.float32) + np.asarray(b1, np.float32), 0.0
        )
        return (h1 @ np.asarray(W2, np.float32) + np.asarray(b2, np.float32))[:, 0]

    state = mlp(inputs["sq_W1"], inputs["sq_b1"], inputs["sq_W2"], inputs["sq_b2"])
    rel = mlp(inputs["rq_W1"], inputs["rq_b1"], inputs["rq_W2"], inputs["rq_b2"])
    labels = np.asarray(inputs["segment_labels"]).reshape(L.SEQ)
    sel = np.where(labels <= 3, state, rel).reshape(16, 8)
    out = (1.0 / (1.0 + np.exp(-sel.mean(axis=1)))).astype(np.float32)
    LAST_PHASE_TIMES["tail"] = time.time() - t0
    return out


# revision 2
# speedup vs baseline: 19904.5011x; 19904.5011x over previous
"""Trainium2 Bass kernel for nn_NeSyBase_36575941493336 (moe_routing).

BiLSTM video/text encoders + tiny routing MLP, mapped onto 8 NeuronCores in
two SPMD dispatches:

D1 (input projections, 8 cores): xg = x @ Wx for all four LSTM runs (video
    fwd/bwd, text fwd/bwd), fp16 matmuls with fp32 PSUM accumulation. Work is
    split per core by (time-quarter, seq-half); weights replicated. Outputs
    are stored [t, gate, seq] fp16.

D2 (recurrence, 8 cores): each core runs ONE (direction, seq-half) stream at
    batch 64 for 256 steps, weights stationary (option "gatesT": hidden state
    kept transposed [hid, seq] so no transposes appear in the loop). Text
    streams (T=64) are front-padded with 192 zero-xg steps — with bias folded
    into xg, zero xg keeps (h, c) exactly (0, 0), so the same 256-step SPMD
    program serves both sequence lengths. Gate order along the 4H axis is
    host-permuted to [g | i | f | o] so the critical o-gate tail is short.

The 0.06%-of-FLOPs routing MLP + segment select + mean + sigmoid runs on the
host in fp32.

Self-contained: only needs numpy + the concourse/bass toolchain.
"""

import numpy as np

import concourse.bacc as bacc
import concourse.mybir as mybir
import concourse.tile as tile
from concourse.bass_utils import run_bass_kernel_spmd

FP16 = mybir.dt.float16
FP32 = mybir.dt.float32

SEQ = 128          # B*S sequences
TV = 256           # video timesteps
TT = 64            # text timesteps
DV = 1024          # video input dim (2E)
DT = 512           # text input dim (E)
H = 512            # hidden
G = 2048           # 4H gates
SH = 64            # seqs per half
ACT = mybir.ActivationFunctionType


def _new_nc():
    return bacc.Bacc("TRN2", target_bir_lowering=False)


# --------------------------------------------------------------------------
# D1: input projections
# --------------------------------------------------------------------------

def build_d1(tqv=TV // 4, tqt=TT // 4):
    """Per core: xg for video f/b over [tqv timesteps x 64 seqs] and text f/b
    over [tqt x 64]. Inputs are pre-transposed to [D, t*s] fp16 on host."""
    nc = _new_nc()
    xtv = nc.declare_dram_parameter("xtv", [DV, tqv * SH], FP16, isOutput=False)
    xtt = nc.declare_dram_parameter("xtt", [DT, tqt * SH], FP16, isOutput=False)
    wvf = nc.declare_dram_parameter("wvf", [DV, G], FP16, isOutput=False)
    wvb = nc.declare_dram_parameter("wvb", [DV, G], FP16, isOutput=False)
    wtf = nc.declare_dram_parameter("wtf", [DT, G], FP16, isOutput=False)
    wtb = nc.declare_dram_parameter("wtb", [DT, G], FP16, isOutput=False)
    xgvf = nc.declare_dram_parameter("xgvf", [tqv, G, SH], FP16, isOutput=True)
    xgvb = nc.declare_dram_parameter("xgvb", [tqv, G, SH], FP16, isOutput=True)
    xgtf = nc.declare_dram_parameter("xgtf", [tqt, G, SH], FP16, isOutput=True)
    xgtb = nc.declare_dram_parameter("xgtb", [tqt, G, SH], FP16, isOutput=True)

    with tile.TileContext(nc) as tc:
        with (
            tc.tile_pool(name="xin", bufs=1) as xin,
            tc.tile_pool(name="win", bufs=2) as win,
            tc.tile_pool(name="ps", bufs=8, space="PSUM") as psp,
            tc.tile_pool(name="out", bufs=6) as outp,
        ):
            # resident activations (shared by fwd/bwd of each modality)
            xv_sb = xin.tile([128, DV // 128, tqv * SH], FP16, tag="xv")
            nc.sync.dma_start(out=xv_sb, in_=xtv.rearrange("(kt p) n -> p kt n", p=128))
            xt_sb = xin.tile([128, DT // 128, tqt * SH], FP16, tag="xt")
            nc.sync.dma_start(out=xt_sb, in_=xtt.rearrange("(kt p) n -> p kt n", p=128))

            def project(w_dram, x_sb, xg_dram, kt, ncols, wtag):
                w_sb = win.tile([128, kt, G], FP16, tag=wtag, name=f"w_{wtag}")
                nc.sync.dma_start(
                    out=w_sb, in_=w_dram.rearrange("(kt p) g -> p kt g", p=128)
                )
                nchunk = ncols // 512
                tpc = 512 // SH  # timesteps covered per 512-col chunk
                for n in range(nchunk):
                    for m in range(16):
                        ps = psp.tile([128, 512], FP32, tag="ps", name="ps")
                        for k in range(kt):
                            nc.tensor.matmul(
                                ps,
                                lhsT=w_sb[:, k, 128 * m : 128 * (m + 1)],
                                rhs=x_sb[:, k, 512 * n : 512 * (n + 1)],
                                start=(k == 0),
                                stop=(k == kt - 1),
                            )
                        ot = outp.tile([128, 512], FP16, tag="ot", name="ot")
                        nc.scalar.copy(out=ot, in_=ps)
                        dst = xg_dram[
                            n * tpc : (n + 1) * tpc, 128 * m : 128 * (m + 1), :
                        ].rearrange("t g s -> g t s")
                        nc.sync.dma_start(
                            out=dst, in_=ot.rearrange("p (t s) -> p t s", t=tpc)
                        )

            project(wvf, xv_sb, xgvf, DV // 128, tqv * SH, "wv")
            project(wvb, xv_sb, xgvb, DV // 128, tqv * SH, "wv")
            project(wtf, xt_sb, xgtf, DT // 128, tqt * SH, "wt")
            project(wtb, xt_sb, xgtb, DT // 128, tqt * SH, "wt")

    nc.compile()
    return nc


# --------------------------------------------------------------------------
# D2: LSTM recurrence, one (direction, seq-half) stream per core, batch 64
# --------------------------------------------------------------------------

def build_d2(T=TV):
    """Gate order along G is host-permuted to [g | i | f | o] (see
    permute_gates): g first so the c-chain runs under later matmuls, o last
    so the critical tail is just pre_o -> sigmoid -> h_mul."""
    nc = _new_nc()
    wh = nc.declare_dram_parameter("wh", [H, G], FP16, isOutput=False)
    xg = nc.declare_dram_parameter("xg", [T, G, SH], FP16, isOutput=False)
    hout = nc.declare_dram_parameter(
        "hout", [128, (H // 128) * SH], FP32, isOutput=True
    )

    KT = H // 128  # 4 k-tiles
    BL = (H // 128) * SH  # 256: block-layout free size for states
    GB = 4 * SH  # 256: one gate-type block

    with tile.TileContext(nc) as tc:
        with (
            tc.tile_pool(name="w", bufs=1) as wp,
            tc.tile_pool(name="xg", bufs=6) as xgp,
            tc.tile_pool(name="ps", bufs=2, space="PSUM") as psp,
            tc.tile_pool(name="ew", bufs=3) as ewp,
            tc.tile_pool(name="st", bufs=3) as stp,
        ):
            wh_sb = wp.tile([128, KT, G], FP16, tag="wh")
            nc.sync.dma_start(out=wh_sb, in_=wh.rearrange("(kt p) g -> p kt g", p=128))

            hT = stp.tile([128, BL], FP16, tag="h")
            nc.vector.memset(hT, 0.0)
            cT = stp.tile([128, BL], FP32, tag="c")
            nc.vector.memset(cT, 0.0)

            def mm_group(ps, mlo, mhi, h_rhs):
                for j in range(mhi - mlo):
                    m = mlo + j
                    for k in range(KT):
                        nc.tensor.matmul(
                            ps[:, SH * j : SH * (j + 1)],
                            lhsT=wh_sb[:, k, 128 * m : 128 * (m + 1)],
                            rhs=h_rhs[:, SH * k : SH * (k + 1)],
                            start=(k == 0),
                            stop=(k == KT - 1),
                        )

            for t in range(T):
                xg_t = xgp.tile([128, 16, SH], FP16, tag="xg")
                nc.sync.dma_start(
                    out=xg_t, in_=xg[t].rearrange("(m p) s -> p m s", p=128)
                )

                def xg_sl(mlo, mhi):
                    return xg_t[:, mlo:mhi, :].rearrange("p m s -> p (m s)")

                # psum tiles: g [1 bank], i|f [1 bank], o [1 bank]
                psG = psp.tile([128, GB], FP32, tag="psG", name="psG")
                psIF = psp.tile([128, 2 * GB], FP32, tag="psIF", name="psIF")
                psO = psp.tile([128, GB], FP32, tag="psO", name="psO")

                h_prev = hT
                mm_group(psG, 0, 4, h_prev)
                mm_group(psIF, 4, 12, h_prev)

                pre_g = ewp.tile([128, GB], FP32, tag="pre_g")
                nc.vector.tensor_add(pre_g, psG, xg_sl(0, 4))
                tanh_g = ewp.tile([128, GB], FP32, tag="tanh_g")
                nc.scalar.activation(tanh_g, pre_g, ACT.Tanh)

                mm_group(psO, 12, 16, h_prev)

                pre_if = ewp.tile([128, 2 * GB], FP32, tag="pre_if")
                nc.vector.tensor_add(pre_if, psIF, xg_sl(4, 12))
                sig_if = ewp.tile([128, 2 * GB], FP32, tag="sig_if")
                nc.scalar.activation(sig_if, pre_if, ACT.Sigmoid)

                ig = ewp.tile([128, BL], FP32, tag="ig")
                nc.vector.tensor_mul(ig, sig_if[:, 0:GB], tanh_g)
                fc = ewp.tile([128, BL], FP32, tag="fc")
                nc.vector.tensor_mul(fc, sig_if[:, GB : 2 * GB], cT)
                cT = stp.tile([128, BL], FP32, tag="c")
                nc.vector.tensor_add(cT, fc, ig)
                tanh_c = ewp.tile([128, BL], FP32, tag="tanh_c")
                nc.scalar.activation(tanh_c, cT, ACT.Tanh)

                pre_o = ewp.tile([128, GB], FP32, tag="pre_o")
                nc.vector.tensor_add(pre_o, psO, xg_sl(12, 16))
                sig_o = ewp.tile([128, GB], FP32, tag="sig_o")
                nc.scalar.activation(sig_o, pre_o, ACT.Sigmoid)

                hT = stp.tile([128, BL], FP16, tag="h")
                nc.vector.tensor_mul(hT, sig_o, tanh_c)

                if t == T - 1:
                    hF = stp.tile([128, BL], FP32, tag="hf")
                    nc.vector.tensor_mul(hF, sig_o, tanh_c)
                    nc.sync.dma_start(out=hout[:, :], in_=hF)

    nc.compile()
    return nc


# --------------------------------------------------------------------------
# Host orchestration
# --------------------------------------------------------------------------

def permute_gates(w):
    """[.., 4H] in torch order [i|f|g|o] -> kernel order [g|i|f|o]."""
    i, f, g, o = np.split(np.asarray(w), 4, axis=-1)
    return np.concatenate([g, i, f, o], axis=-1)


def _prep_d1_inputs(vid, txt, wxvf, wxvb, wxtf, wxtb):
    """vid: [SEQ, TV, DV] f32, txt: [SEQ, TT, DT] f32 -> per-core in_maps."""
    tqv, tqt = TV // 4, TT // 4
    w16 = {
        "wvf": permute_gates(wxvf).astype(np.float16),
        "wvb": permute_gates(wxvb).astype(np.float16),
        "wtf": permute_gates(wxtf).astype(np.float16),
        "wtb": permute_gates(wxtb).astype(np.float16),
    }
    in_maps = []
    for c in range(8):
        q, h = c % 4, c // 4
        cv = vid[SH * h : SH * (h + 1), tqv * q : tqv * (q + 1), :]  # [64, tqv, DV]
        ct = txt[SH * h : SH * (h + 1), tqt * q : tqt * (q + 1), :]
        xtv = np.ascontiguousarray(cv.transpose(2, 1, 0)).reshape(DV, -1)
        xtt = np.ascontiguousarray(ct.transpose(2, 1, 0)).reshape(DT, -1)
        in_maps.append(
            {"xtv": xtv.astype(np.float16), "xtt": xtt.astype(np.float16), **w16}
        )
    return in_maps


def _assemble_d2_inputs(d1_results, whvf, whvb, whtf, whtb):
    """Regroup D1 per-core xg chunks into per-D2-core [256, G, 64] streams."""

    def cat(key, h):
        return np.concatenate([d1_results[h * 4 + q][key] for q in range(4)], axis=0)

    pad = np.zeros((TV - TT, G, SH), np.float16)
    whvf, whvb = permute_gates(whvf), permute_gates(whvb)
    whtf, whtb = permute_gates(whtf), permute_gates(whtb)
    whs = {0: whvf, 1: whvf, 2: whvb, 3: whvb, 4: whtf, 5: whtf, 6: whtb, 7: whtb}
    in_maps = []
    for c in range(8):
        h = c % 2
        if c < 2:
            xg_full = cat("xgvf", h)
        elif c < 4:
            xg_full = cat("xgvb", h)[::-1]
        elif c < 6:
            xg_full = np.concatenate([pad, cat("xgtf", h)], axis=0)
        else:
            xg_full = np.concatenate([pad, cat("xgtb", h)[::-1]], axis=0)
        in_maps.append(
            {"wh": whs[c].astype(np.float16), "xg": np.ascontiguousarray(xg_full)}
        )
    return in_maps


def _assemble_feats(d2_results):
    """d2 core outputs [128, 4*64] -> feats [SEQ, 4H] (vf | vb | tf | tb)."""
    feats = np.zeros((SEQ, 4 * H), np.float32)
    for c in range(8):
        h = c % 2
        d = c // 2  # 0 vf, 1 vb, 2 tf, 3 tb
        hT = d2_results[c]["hout"]  # [128, 4*64]
        for k in range(H // 128):
            blk = hT[:, SH * k : SH * (k + 1)]  # [128 hid, 64 seq]
            feats[
                SH * h : SH * (h + 1), d * H + 128 * k : d * H + 128 * (k + 1)
            ] = blk.T
    return feats


_CACHE = {}
LAST_PHASE_TIMES = {}


def kernel(**inputs) -> np.ndarray:
    import time

    if "d1" not in _CACHE:
        _CACHE["d1"] = build_d1()
        _CACHE["d2"] = build_d2()
    d1_nc, d2_nc = _CACHE["d1"], _CACHE["d2"]

    vid = np.asarray(inputs["vid_feats"], np.float32).reshape(SEQ, TV, DV)
    txt = np.asarray(inputs["text_feats"], np.float32).reshape(SEQ, TT, DT)

    # LSTM biases are zeros in this problem; the kernel folds biases into xg
    # implicitly only when they are zero.
    for bname in ("vid_b_f", "vid_b_b", "txt_b_f", "txt_b_b"):
        assert not np.any(np.asarray(inputs[bname])), (
            f"nonzero LSTM bias {bname} not supported"
        )

    t0 = time.time()
    d1_in = _prep_d1_inputs(
        vid, txt,
        np.asarray(inputs["vid_Wx_f"]), np.asarray(inputs["vid_Wx_b"]),
        np.asarray(inputs["txt_Wx_f"]), np.asarray(inputs["txt_Wx_b"]),
    )
    LAST_PHASE_TIMES["prep_d1"] = time.time() - t0

    t0 = time.time()
    r1 = run_bass_kernel_spmd(d1_nc, d1_in, list(range(8)))
    LAST_PHASE_TIMES["d1"] = time.time() - t0

    t0 = time.time()
    d2_in = _assemble_d2_inputs(
        r1.results,
        np.asarray(inputs["vid_Wh_f"]), np.asarray(inputs["vid_Wh_b"]),
        np.asarray(inputs["txt_Wh_f"]), np.asarray(inputs["txt_Wh_b"]),
    )
    LAST_PHASE_TIMES["prep_d2"] = time.time() - t0

    t0 = time.time()
    r2 = run_bass_kernel_spmd(d2_nc, d2_in, list(range(8)))
    LAST_PHASE_TIMES["d2"] = time.time() - t0

    t0 = time.time()
    feats = _assemble_feats(r2.results)

    def mlp(W1, b1, W2, b2):
        h1 = np.maximum(
            feats @ np.asarray(W1, np.float32) + np.asarray(b1, np.float32), 0.0
        )
        return (h1 @ np.asarray(W2, np.float32) + np.asarray(b2, np.float32))[:, 0]

    state = mlp(inputs["sq_W1"], inputs["sq_b1"], inputs["sq_W2"], inputs["sq_b2"])
    rel = mlp(inputs["rq_W1"], inputs["rq_b1"], inputs["rq_W2"], inputs["rq_b2"])
    labels = np.asarray(inputs["segment_labels"]).reshape(SEQ)
    sel = np.where(labels <= 3, state, rel).reshape(16, 8)
    out = (1.0 / (1.0 + np.exp(-sel.mean(axis=1)))).astype(np.float32)
    LAST_PHASE_TIMES["tail"] = time.time() - t0
    return out
